# revision 1
# baseline (speedup 1.0000x reference)
"""Trainium2 Bass kernel for a 3-layer Lorentz (hyperboloid) MLP.

Math: the reference chains lorentz_linear + inter-layer projx(expmap0(logmap0(.))).
Algebraically, expmap0 -> projx -> logmap0 round-trips cancel: the inter-layer op
on the tangent vector y is exactly "zero the time component, clamp the row norm
of y[1:] to 10".  So the network is:

  t0 = logmap0(x)                       (row scale d/||xs|| on xs, time comp 0)
  y1 = t0 @ W1.T + b1 ; t1 = clamp(y1)  (zero col 0, clamp row norm to 10)
  y2 = t1 @ W2.T + b2 ; t2 = clamp(y2)
  y3 = t2 @ W3.T + b3
  out = [cosh(nc), sinh(nc)/n * y3[1:]] with n=clip(||y3[1:]||,eps), nc=min(n,10)

With zero biases (the shipped case), per-token scales commute through the
GEMMs, so all clamp/logmap scales are folded into one cumulative per-token
scale applied once at the very end ("fold" mode) - the PE runs the three
GEMMs back to back with no inter-layer barrier.  With nonzero biases a
general barrier path (scale applied between layers) is built instead.

Layout: everything on-chip is FEATURE-major ([feat, token]); weights are
pre-transposed/blocked/bf16-cast on the host so each m-tile loads with one
fully contiguous DMA.  Row-wise (per-token) norms are ones-vector matmuls on
the TensorEngine (partition-dim reduction), pipelined one m-tile behind the
main GEMM stream; per-token scales broadcast across partitions with
gpsimd.partition_broadcast.

Sharding: pure data-parallel over tokens - 8192 tokens -> 8 cores x 1024.
"""

import os
import sys
import functools

import numpy as np
import ml_dtypes


def _import_concourse():
    try:
        import concourse  # noqa: F401
    except ImportError:
        for p in ("/opt/trn_rl_repo", "/root/.axon_site/_ro/trn_rl_repo"):
            if os.path.isdir(p) and p not in sys.path:
                sys.path.insert(0, p)
        import concourse  # noqa: F401


_import_concourse()

import concourse.bass as bass  # noqa: E402,F401
import concourse.bacc as bacc  # noqa: E402
import concourse.mybir as mybir  # noqa: E402
import concourse.tile as tile  # noqa: E402
from concourse import bass_utils  # noqa: E402

F32 = mybir.dt.float32
BF16 = mybir.dt.bfloat16
AF = mybir.ActivationFunctionType
ALU = mybir.AluOpType

P = 128
N_CORES = 8
EPS = 1e-7
MAX_TAN_NORM = 10.0

# Full-problem dims (hardcoded per spec)
TOK, D_IN, D_HID, D_OUT = 8192, 1024, 4096, 1024
TOKPC = TOK // N_CORES  # tokens per core


def build_nc(tokpc=TOKPC, din=D_IN, dhid=D_HID, dout=D_OUT, ch=512,
             repeat=1, fold=False):
    """Build + compile the per-core Bass program."""
    assert tokpc % ch == 0
    nch = tokpc // ch
    kt1, mt1 = din // P, dhid // P
    kt2, mt2 = dhid // P, dhid // P
    kt3, mt3 = dhid // P, dout // P

    nc = bacc.Bacc("TRN2", target_bir_lowering=False, debug=False,
                   num_devices=N_CORES)

    xt_d = nc.dram_tensor("xt", [din, tokpc], BF16, kind="ExternalInput")
    x0_d = nc.dram_tensor("x0", [1, tokpc], F32, kind="ExternalInput")
    w1_d = nc.dram_tensor("w1", [mt1 * P, din], BF16, kind="ExternalInput")
    w2_d = nc.dram_tensor("w2", [mt2 * P, dhid], BF16, kind="ExternalInput")
    w3_d = nc.dram_tensor("w3", [mt3 * P, dhid], BF16, kind="ExternalInput")
    b1_d = nc.dram_tensor("b1", [P, mt1], F32, kind="ExternalInput")
    b2_d = nc.dram_tensor("b2", [P, mt2], F32, kind="ExternalInput")
    b3_d = nc.dram_tensor("b3", [P, mt3], F32, kind="ExternalInput")
    out_d = nc.dram_tensor("out", [dout, tokpc], F32, kind="ExternalOutput")

    with tile.TileContext(nc) as tc:
        _build_tile_program(tc, nc, dict(
            tokpc=tokpc, din=din, dhid=dhid, dout=dout, ch=ch, nch=nch,
            kt1=kt1, mt1=mt1, kt2=kt2, mt2=mt2, kt3=kt3, mt3=mt3,
            xt=xt_d, x0=x0_d, w1=w1_d, w2=w2_d, w3=w3_d,
            b1=b1_d, b2=b2_d, b3=b3_d, out=out_d,
        ), repeat=repeat, fold=fold)
    nc.compile()
    return nc


def _build_tile_program(tc, nc, C, repeat=1, fold=False):
    tokpc, ch, nch = C["tokpc"], C["ch"], C["nch"]

    # long-lived pools
    const = tc.alloc_tile_pool(name="const", bufs=1)
    scalL = tc.alloc_tile_pool(name="scalL", bufs=5)   # [1, tokpc] f32
    scalS = tc.alloc_tile_pool(name="scalS", bufs=6)   # [1, ch] f32
    bcast = tc.alloc_tile_pool(name="bcast", bufs=1 if fold else 2)
    sqp = tc.alloc_tile_pool(name="sq", bufs=2)
    accp = tc.alloc_tile_pool(name="acc", bufs=4)
    wp = tc.alloc_tile_pool(name="wt", bufs=3)
    psy = tc.alloc_tile_pool(name="psy", bufs=4, space="PSUM")
    psn = tc.alloc_tile_pool(name="psn", bufs=4, space="PSUM")
    outp = tc.alloc_tile_pool(name="outp", bufs=4)

    ones_k = const.tile([P, 1], BF16, tag="ones_k")
    nc.vector.memset(ones_k[:], 1.0)
    bias1 = const.tile([P, C["mt1"]], F32, tag="bias1")
    nc.sync.dma_start(bias1[:], C["b1"].ap())
    bias2 = const.tile([P, C["mt2"]], F32, tag="bias2")
    nc.sync.dma_start(bias2[:], C["b2"].ap())
    bias3 = const.tile([P, C["mt3"]], F32, tag="bias3")
    nc.sync.dma_start(bias3[:], C["b3"].ap())

    def stile_l():
        return scalL.tile([1, tokpc], F32, tag="sl", name="sl")

    def stile_s():
        return scalS.tile([1, ch], F32, tag="ss", name="ss")

    def norm_accum_tiles():
        return [psn.tile([1, ch], F32, tag="psn", name=f"psn{_}")
                for _ in range(nch)]

    def bcast_full(s_full):
        sb = bcast.tile([P, tokpc], F32, tag="sb", name="sb")
        nc.gpsimd.partition_broadcast(sb[:], s_full[:])
        return sb

    # ---------------- GEMM layer (layers 1, 2) ----------------
    ones_f = const.tile([P, 1], F32, tag="ones_f", name="ones_f")
    nc.vector.memset(ones_f[:], 1.0)

    def gemm_layer(tin, w_d, bias_t, kt, mt, out_pool, out_dtype, tag,
                   mid_fn=None):
        """y[m] = sum_k w[k,m].T @ tin[k]; ACT evicts (+bias) and squares
        straight from PSUM; squares accumulate on the idle DVE (f32) and a
        single fp32 ones-matmul per chunk does the final partition-reduce.
        mid_fn (emitted after m==1's matmuls) lets the caller defer the
        PREVIOUS layer's norm matmul + scalar chain into this layer's PE
        stream so the PE never stalls on them.  Returns (tiles, finish) -
        call finish() later to emit this layer's norm reduce."""
        accs = [accp.tile([P, ch], F32, tag="acc", name=f"acc{_}")
                for _ in range(nch)]
        tout = []
        for m in range(mt):
            wm = wp.tile([P, kt * P], BF16, tag="wtile", name="wm")
            nc.sync.dma_start(wm[:], w_d.ap()[m * P:(m + 1) * P, :])
            pss = [psy.tile([P, ch], F32, tag="psy", name=f"psy{_}")
                   for _ in range(nch)]
            for k in range(kt):
                for c in range(nch):
                    nc.tensor.matmul(pss[c][:], wm[:, k * P:(k + 1) * P],
                                     tin[k][:, c * ch:(c + 1) * ch],
                                     start=(k == 0), stop=(k == kt - 1))
            if m == 1 and mid_fn is not None:
                mid_fn()
            ty = out_pool.tile([P, tokpc], out_dtype, tag=f"{tag}{m}",
                               name=f"{tag}{m}")
            for c in range(nch):
                nc.scalar.activation(ty[:, c * ch:(c + 1) * ch], pss[c][:],
                                     AF.Identity, bias=bias_t[:, m:m + 1],
                                     scale=1.0)
                if m == 0:
                    nc.scalar.activation(accs[c][:], pss[c][:], AF.Square,
                                         bias=bias_t[:, m:m + 1], scale=1.0)
                    nc.vector.memset(accs[c][0:1, :], 0.0)
                else:
                    sq = sqp.tile([P, ch], F32, tag="sq", name="sq")
                    nc.scalar.activation(sq[:], pss[c][:], AF.Square,
                                         bias=bias_t[:, m:m + 1], scale=1.0)
                    nc.vector.tensor_tensor(accs[c][:], accs[c][:], sq[:],
                                            ALU.add)
            if m == 0:
                nc.vector.memset(ty[0:1, :], 0.0)
            tout.append(ty)

        def finish():
            ps_norm = norm_accum_tiles()
            for c in range(nch):
                nc.tensor.matmul(ps_norm[c][:], ones_f[:], accs[c][:],
                                 start=True, stop=True)
            return ps_norm
        return tout, finish

    def clamp_scale(ps_norm):
        """Barrier path: s = min(max(sqrt(ssq),eps),10)/max(sqrt(ssq),eps)."""
        s = stile_l()
        for c in range(nch):
            n_ = stile_s()
            nc.scalar.activation(n_[:], ps_norm[c][:], AF.Sqrt)
            nc.vector.tensor_scalar_max(n_[:], n_[:], EPS)
            num = stile_s()
            nc.vector.tensor_scalar_min(num[:], n_[:], MAX_TAN_NORM)
            rec = stile_s()
            nc.vector.reciprocal(rec[:], n_[:])
            nc.vector.tensor_tensor(s[:, c * ch:(c + 1) * ch], num[:], rec[:],
                                    ALU.mult)
        return s

    def apply_scale(tiles, sb):
        for t in tiles:
            nc.vector.tensor_tensor(t[:], t[:], sb[:], ALU.mult)

    def clamp_chain(ps_norm, cs_prev):
        """Fold path: true norm n = max(cs_prev*sqrt(ssq_raw), eps);
        cs_new = cs_prev*min(n,10)/n.  [1,tokpc] ops only - nothing on the
        GEMM critical path waits on this."""
        cs_new = stile_l()
        for c in range(nch):
            sl = slice(c * ch, (c + 1) * ch)
            n_ = stile_s()
            nc.scalar.activation(n_[:], ps_norm[c][:], AF.Sqrt)
            nc.vector.tensor_tensor(n_[:], n_[:], cs_prev[:, sl], ALU.mult)
            nc.vector.tensor_scalar_max(n_[:], n_[:], EPS)
            num = stile_s()
            nc.vector.tensor_scalar_min(num[:], n_[:], MAX_TAN_NORM)
            rec = stile_s()
            nc.vector.reciprocal(rec[:], n_[:])
            nc.vector.tensor_tensor(rec[:], num[:], rec[:], ALU.mult)
            nc.vector.tensor_tensor(cs_new[:, sl], cs_prev[:, sl], rec[:],
                                    ALU.mult)
        return cs_new

    def body():
        # ---------- Phase 0: load bf16 xs (= raw t0), input norm, s0 ----------
        t0p = tc.alloc_tile_pool(name="t0", bufs=1, side="right")
        t0 = []
        for k in range(C["kt1"]):
            t = t0p.tile([P, tokpc], BF16, tag=f"t0_{k}", name=f"t0_{k}")
            nc.sync.dma_start(t[:], C["xt"].ap()[k * P:(k + 1) * P, :])
            t0.append(t)
        nc.vector.memset(t0[0][0:1, :], 0.0)  # zero time component
        x0t = stile_l()
        nc.sync.dma_start(x0t[:], C["x0"].ap())

        acc0 = [accp.tile([P, ch], F32, tag="acc", name=f"acc0_{_}")
                for _ in range(nch)]
        for k in range(C["kt1"]):
            for c in range(nch):
                if k == 0:
                    nc.scalar.activation(acc0[c][:],
                                         t0[k][:, c * ch:(c + 1) * ch],
                                         AF.Square)
                else:
                    sq = sqp.tile([P, ch], F32, tag="sq", name="sq")
                    nc.scalar.activation(sq[:], t0[k][:, c * ch:(c + 1) * ch],
                                         AF.Square)
                    nc.vector.tensor_tensor(acc0[c][:], acc0[c][:], sq[:],
                                            ALU.add)

        S = {}

        def phase0_tail():
            # norm-MM for the input + s0 = arccosh(x0) / ||xs|| chain
            ps_n0 = norm_accum_tiles()
            for c in range(nch):
                nc.tensor.matmul(ps_n0[c][:], ones_f[:], acc0[c][:],
                                 start=True, stop=True)
            xc = stile_l()
            nc.vector.tensor_scalar_max(xc[:], x0t[:], 1.0 + EPS)
            t2_ = stile_l()
            nc.vector.tensor_tensor(t2_[:], xc[:], xc[:], ALU.mult)
            nc.vector.tensor_scalar_add(t2_[:], t2_[:], -1.0)
            r_ = stile_l()
            nc.scalar.activation(r_[:], t2_[:], AF.Sqrt)
            nc.vector.tensor_tensor(r_[:], xc[:], r_[:], ALU.add)
            d_ = stile_l()
            nc.scalar.activation(d_[:], r_[:], AF.Ln)
            s0 = stile_l()
            for c in range(nch):
                n_ = stile_s()
                nc.scalar.activation(n_[:], ps_n0[c][:], AF.Sqrt)
                nc.vector.tensor_scalar_max(n_[:], n_[:], EPS)
                rec = stile_s()
                nc.vector.reciprocal(rec[:], n_[:])
                nc.vector.tensor_tensor(s0[:, c * ch:(c + 1) * ch],
                                        d_[:, c * ch:(c + 1) * ch], rec[:],
                                        ALU.mult)
            S["cs"] = s0
            return s0

        if not fold:
            # scale t0 in place before GEMM1 (squares above read pre-scale
            # values; Tile's WAR deps order the in-place multiply after them)
            s0b = bcast_full(phase0_tail())
            for k in range(C["kt1"]):
                nc.vector.tensor_tensor(t0[k][:], t0[k][:], s0b[:], ALU.mult)

        # ---------- Layers 1, 2 (norm chains deferred into the next
        # layer's PE stream in fold mode) ----------
        t1p = tc.alloc_tile_pool(name="t1", bufs=1, side="left")
        t1, fin1 = gemm_layer(t0, C["w1"], bias1, C["kt1"], C["mt1"],
                              t1p, BF16, "t1_",
                              mid_fn=phase0_tail if fold else None)
        if not fold:
            apply_scale(t1, bcast_full(clamp_scale(fin1())))
        t0p.release()

        def l1_tail():
            S["cs"] = clamp_chain(fin1(), S["cs"])

        t2p = tc.alloc_tile_pool(name="t2", bufs=1, side="right")
        t2, fin2 = gemm_layer(t1, C["w2"], bias2, C["kt2"], C["mt2"],
                              t2p, BF16, "t2_",
                              mid_fn=l1_tail if fold else None)
        if not fold:
            apply_scale(t2, bcast_full(clamp_scale(fin2())))
        t1p.release()

        def l2_tail():
            S["cs"] = clamp_chain(fin2(), S["cs"])

        # ---------- Layer 3 + expmap0/projx, chunk-split so chunk 0's tail
        # overlaps chunk 1's matmuls ----------
        kt, mt = C["kt3"], C["mt3"]
        y3p = tc.alloc_tile_pool(name="y3", bufs=1, side="left")
        y3 = [y3p.tile([P, tokpc], F32, tag=f"y3_{m}", name=f"y3_{m}")
              for m in range(mt)]
        deferred_tail = None
        for c in range(nch):
            sl = slice(c * ch, (c + 1) * ch)
            acc3 = accp.tile([P, ch], F32, tag="acc", name=f"acc3_{c}")
            for m in range(mt):
                wm = wp.tile([P, kt * P], BF16, tag="wtile", name="wm")
                nc.sync.dma_start(wm[:], C["w3"].ap()[m * P:(m + 1) * P, :])
                ps = psy.tile([P, ch], F32, tag="psy", name="psy3")
                for k in range(kt):
                    nc.tensor.matmul(ps[:], wm[:, k * P:(k + 1) * P],
                                     t2[k][:, sl],
                                     start=(k == 0), stop=(k == kt - 1))
                if m == 1:
                    if c == 0 and fold:
                        l2_tail()
                    if deferred_tail is not None:
                        deferred_tail()
                        deferred_tail = None
                nc.scalar.activation(y3[m][:, sl], ps[:], AF.Identity,
                                     bias=bias3[:, m:m + 1], scale=1.0)
                if m == 0:
                    nc.scalar.activation(acc3[:], ps[:], AF.Square,
                                         bias=bias3[:, m:m + 1], scale=1.0)
                    nc.vector.memset(acc3[0:1, :], 0.0)
                else:
                    sq = sqp.tile([P, ch], F32, tag="sq", name="sq")
                    nc.scalar.activation(sq[:], ps[:], AF.Square,
                                         bias=bias3[:, m:m + 1], scale=1.0)
                    nc.vector.tensor_tensor(acc3[:], acc3[:], sq[:], ALU.add)

            def chunk_tail(sl=sl, acc3=acc3, c=c):
                # n=max(.,eps); ncl=min(n,10); s3=[cs*]sinh(ncl)/n; x0=cosh
                ps_norm = psn.tile([1, ch], F32, tag="psn", name=f"psn3_{c}")
                nc.tensor.matmul(ps_norm[:], ones_f[:], acc3[:],
                                 start=True, stop=True)
                n_ = stile_s()
                nc.scalar.activation(n_[:], ps_norm[:], AF.Sqrt)
                if fold:
                    nc.vector.tensor_tensor(n_[:], n_[:], S["cs"][:, sl],
                                            ALU.mult)
                nc.vector.tensor_scalar_max(n_[:], n_[:], EPS)
                ncl = stile_s()
                nc.vector.tensor_scalar_min(ncl[:], n_[:], MAX_TAN_NORM)
                e_ = stile_s()
                nc.scalar.activation(e_[:], ncl[:], AF.Exp)
                nn = stile_s()
                nc.vector.tensor_scalar_mul(nn[:], ncl[:], -1.0)
                en = stile_s()
                nc.scalar.activation(en[:], nn[:], AF.Exp)
                sh = stile_s()
                nc.vector.tensor_tensor(sh[:], e_[:], en[:], ALU.subtract)
                nc.vector.tensor_scalar_mul(sh[:], sh[:], 0.5)
                cosh_c = stile_s()
                nc.vector.tensor_tensor(cosh_c[:], e_[:], en[:], ALU.add)
                nc.vector.tensor_scalar_mul(cosh_c[:], cosh_c[:], 0.5)
                rec = stile_s()
                nc.vector.reciprocal(rec[:], n_[:])
                s3 = stile_s()
                nc.vector.tensor_tensor(s3[:], sh[:], rec[:], ALU.mult)
                if fold:
                    nc.vector.tensor_tensor(s3[:], s3[:], S["cs"][:, sl],
                                            ALU.mult)
                s3b = bcast.tile([P, ch], F32, tag="s3b", name="s3b")
                nc.gpsimd.partition_broadcast(s3b[:], s3[:])
                for m in range(mt):
                    ot = outp.tile([P, ch], F32, tag="ot", name="ot")
                    eng = nc.vector if m % 2 == 0 else nc.gpsimd
                    eng.tensor_tensor(ot[:], y3[m][:, sl], s3b[:], ALU.mult)
                    if m == 0:
                        nc.vector.tensor_copy(ot[0:1, :], cosh_c[:])
                    nc.sync.dma_start(C["out"].ap()[m * P:(m + 1) * P, sl],
                                      ot[:])

            deferred_tail = chunk_tail
        deferred_tail()
        t2p.release()
        y3p.release()

    for _rep in range(repeat):
        body()

    for p in (outp, psn, psy, wp, accp, sqp, bcast, scalS, scalL, const):
        p.release()


# ---------------- host-side prep + entry point ----------------

def _block_weight(w):
    """W [dout, din] f32 -> [mt*128, din] bf16 with row m*128+p holding, for
    each k-tile, lhsT tile (k,m) row p: out[m*128+p, k*128+j] = W.T[k*128+p,
    m*128+j].  One fully-contiguous [128, kt*128] DMA per m-tile."""
    dout, din = w.shape
    mt, kt = dout // P, din // P
    w = np.asarray(w, dtype=np.float32)
    blocked = (w.reshape(mt, P, kt, P)       # [m, j, k, p]
                .transpose(0, 3, 2, 1)       # [m, p, k, j]
                .reshape(mt * P, din))
    return np.ascontiguousarray(blocked.astype(ml_dtypes.bfloat16))


def _prep_bias(b, mt):
    """b [d] -> [128, mt] f32 with out[p, m] = b[m*128+p]."""
    return np.ascontiguousarray(
        np.asarray(b, dtype=np.float32).reshape(mt, P).T)


@functools.lru_cache(maxsize=2)
def _get_nc(fold=False):
    return build_nc(fold=fold)


def prep_in_maps(x_hyp, W1, b1, W2, b2, W3, b3):
    w1b = _block_weight(W1)
    w2b = _block_weight(W2)
    w3b = _block_weight(W3)
    b1c = _prep_bias(b1, D_HID // P)
    b2c = _prep_bias(b2, D_HID // P)
    b3c = _prep_bias(b3, D_OUT // P)
    x = np.asarray(x_hyp, dtype=np.float32)
    in_maps = []
    for c in range(N_CORES):
        shard = x[c * TOKPC:(c + 1) * TOKPC, :]  # [tokpc, din]
        xt = np.ascontiguousarray(shard.T.astype(ml_dtypes.bfloat16))
        x0 = np.ascontiguousarray(shard[:, 0:1].T)  # [1, tokpc] f32
        in_maps.append(dict(xt=xt, x0=x0, w1=w1b, w2=w2b, w3=w3b,
                            b1=b1c, b2=b2c, b3=b3c))
    return in_maps


def kernel(x_hyp, W1, b1, W2, b2, W3, b3):
    fold = not (np.any(b1) or np.any(b2) or np.any(b3))
    nc = _get_nc(fold)
    in_maps = prep_in_maps(x_hyp, W1, b1, W2, b2, W3, b3)
    res = bass_utils.run_bass_kernel_spmd(nc, in_maps,
                                          core_ids=list(range(N_CORES)))
    parts = [np.asarray(res.results[c]["out"]).T for c in range(N_CORES)]
    return np.ascontiguousarray(np.concatenate(parts, axis=0),
                                dtype=np.float32)



# revision 6
# speedup vs baseline: 18961.7797x; 18961.7797x over previous
"""Trainium2 Bass kernel for a 3-layer Lorentz (hyperboloid) MLP.

Math: the reference chains lorentz_linear + inter-layer projx(expmap0(logmap0(.))).
Algebraically, expmap0 -> projx -> logmap0 round-trips cancel: the inter-layer op
on the tangent vector y is exactly "zero the time component, clamp the row norm
of y[1:] to 10".  So the network is:

  t0 = logmap0(x)                       (row scale d/||xs|| on xs, time comp 0)
  y1 = t0 @ W1.T + b1 ; t1 = clamp(y1)  (zero col 0, clamp row norm to 10)
  y2 = t1 @ W2.T + b2 ; t2 = clamp(y2)
  y3 = t2 @ W3.T + b3
  out = [cosh(nc), sinh(nc)/n * y3[1:]] with n=clip(||y3[1:]||,eps), nc=min(n,10)

With zero biases (the shipped case), per-token scales commute through the
GEMMs; logmap0 is applied on the host (t0 shipped pre-scaled in bf16) and the
inter-layer clamps are folded into a cumulative SQUARED per-token scale cs2
tracked on [1,tok] vectors, applied once at the very end.  The PE runs the
three GEMMs back to back with no inter-layer barrier.

All scalar-engine transcendentals (Ln, Exp, Identity, Square) live in the
single `natural_log_exp_and_others` table set, so there are no mid-kernel
ACT_TABLE_LOADs; rsqrt/sqrt are computed as exp(+-0.5*ln(x)) which is also
far more accurate than the Sqrt table (65536-ULP budget).

Layout: everything on-chip is FEATURE-major ([feat, token]); weights are
pre-transposed/blocked/bf16-cast on the host so each m-tile loads with one
fully contiguous DMA.  Row-wise (per-token) sum-of-squares are ones-vector
matmuls on the TensorEngine (partition-dim reduction), deferred one m-tile
into the next layer's PE stream so the PE never stalls on them.  GpSimd does
nothing but the two final partition_broadcasts (single ucode lib, loaded once
during the MM stream).

Sharding: pure data-parallel over tokens - 8192 tokens -> 8 cores x 1024.

With nonzero biases a general barrier path (scale applied between layers,
arccosh on device) is kept as fallback.
"""

import math
import os
import sys
import functools

import numpy as np
import ml_dtypes


def _import_concourse():
    try:
        import concourse  # noqa: F401
    except ImportError:
        for p in ("/opt/trn_rl_repo", "/root/.axon_site/_ro/trn_rl_repo"):
            if os.path.isdir(p) and p not in sys.path:
                sys.path.insert(0, p)
        import concourse  # noqa: F401


_import_concourse()

import concourse.bass as bass  # noqa: E402,F401
import concourse.bacc as bacc  # noqa: E402
import concourse.mybir as mybir  # noqa: E402
import concourse.tile as tile  # noqa: E402
from concourse import bass_utils  # noqa: E402

F32 = mybir.dt.float32
BF16 = mybir.dt.bfloat16
AF = mybir.ActivationFunctionType
ALU = mybir.AluOpType

P = 128
N_CORES = 8
EPS = 1e-7
EPS2 = 1e-14  # EPS**2, the ln(ssq + eps^2) guard
MAX_TAN_NORM = 10.0
LN10 = math.log(10.0)
LNHALF = math.log(0.5)

# Full-problem dims (hardcoded per spec)
TOK, D_IN, D_HID, D_OUT = 8192, 1024, 4096, 1024
TOKPC = TOK // N_CORES  # tokens per core


# =====================================================================
# Fold-mode program (zero biases - the shipped case)
# =====================================================================

def build_nc_fold(tokpc=TOKPC, din=D_IN, dhid=D_HID, dout=D_OUT, ch=512):
    assert tokpc % ch == 0
    nch = tokpc // ch
    kt1, mt1 = din // P, dhid // P
    kt2, mt2 = dhid // P, dhid // P
    kt3, mt3 = dhid // P, dout // P

    nc = bacc.Bacc("TRN2", target_bir_lowering=False, debug=False,
                   num_devices=N_CORES)

    xt_d = nc.dram_tensor("xt", [din, tokpc], BF16, kind="ExternalInput")
    w1_d = nc.dram_tensor("w1", [mt1 * P, din], BF16, kind="ExternalInput")
    w2_d = nc.dram_tensor("w2", [mt2 * P, dhid], BF16, kind="ExternalInput")
    w3_d = nc.dram_tensor("w3", [mt3 * P, dhid], BF16, kind="ExternalInput")
    out_d = nc.dram_tensor("out", [dout, tokpc], F32, kind="ExternalOutput")

    with tile.TileContext(nc) as tc:
        _build_fold_program(tc, nc, dict(
            tokpc=tokpc, ch=ch, nch=nch,
            kt1=kt1, mt1=mt1, kt2=kt2, mt2=mt2, kt3=kt3, mt3=mt3,
            xt=xt_d, w1=w1_d, w2=w2_d, w3=w3_d, out=out_d,
        ))
    nc.compile()
    return nc


def _build_fold_program(tc, nc, C):
    tokpc, ch, nch = C["tokpc"], C["ch"], C["nch"]

    const = tc.alloc_tile_pool(name="const", bufs=1)
    scalL = tc.alloc_tile_pool(name="scalL", bufs=2)   # [1, tokpc] f32 (cs2)
    scalS = tc.alloc_tile_pool(name="scalS", bufs=6)   # [1, ch] f32
    bcast = tc.alloc_tile_pool(name="bcast", bufs=2)
    sqp = tc.alloc_tile_pool(name="sq", bufs=2)
    accp = tc.alloc_tile_pool(name="acc", bufs=4)
    wp = tc.alloc_tile_pool(name="wt", bufs=3)
    psy = tc.alloc_tile_pool(name="psy", bufs=6, space="PSUM")
    psn = tc.alloc_tile_pool(name="psn", bufs=2, space="PSUM")
    outp = tc.alloc_tile_pool(name="outp", bufs=4)

    ones_f = const.tile([P, 1], F32, tag="ones_f", name="ones_f")
    nc.vector.memset(ones_f[:], 1.0)
    c_eps2 = const.tile([P, 1], F32, tag="c_eps2", name="c_eps2")
    nc.vector.memset(c_eps2[:], EPS2)
    c_ln10 = const.tile([P, 1], F32, tag="c_ln10", name="c_ln10")
    nc.vector.memset(c_ln10[:], LN10)
    c_lnhalf = const.tile([P, 1], F32, tag="c_lnhalf", name="c_lnhalf")
    nc.vector.memset(c_lnhalf[:], LNHALF)

    # ---- head: first GEMM's gating DMAs go out first ----
    t0p = tc.alloc_tile_pool(name="t0", bufs=1, side="right")
    w1_pre = []
    t0 = []

    wm0 = wp.tile([P, C["kt1"] * P], BF16, tag="wtile", name="wm0")
    nc.sync.dma_start(wm0[:], C["w1"].ap()[0:P, :])
    w1_pre.append(wm0)
    t = t0p.tile([P, tokpc], BF16, tag="t0_0", name="t0_0")
    nc.sync.dma_start(t[:], C["xt"].ap()[0:P, :])
    t0.append(t)
    wm1 = wp.tile([P, C["kt1"] * P], BF16, tag="wtile", name="wm1")
    nc.sync.dma_start(wm1[:], C["w1"].ap()[P:2 * P, :])
    w1_pre.append(wm1)
    for k in range(1, C["kt1"]):
        t = t0p.tile([P, tokpc], BF16, tag=f"t0_{k}", name=f"t0_{k}")
        nc.sync.dma_start(t[:], C["xt"].ap()[k * P:(k + 1) * P, :])
        t0.append(t)

    def stile_s():
        return scalS.tile([1, ch], F32, tag="ss", name="ss")

    S = {"cs2": None}

    def gemm_layer(tin, w_d, kt, mt, out_pool, tag, mid_fn=None,
                   preloaded=()):
        """y[m] = sum_k w[k,m].T @ tin[k]; ACT evicts and squares straight
        from PSUM; squares accumulate on the DVE (f32) and a single fp32
        ones-matmul per chunk (deferred via finish()) does the final
        partition-reduce.  mid_fn is emitted after m==1's matmuls."""
        accs = [accp.tile([P, ch], F32, tag="acc", name=f"acc{_}")
                for _ in range(nch)]
        tout = []
        for m in range(mt):
            if m < len(preloaded):
                wm = preloaded[m]
            else:
                wm = wp.tile([P, kt * P], BF16, tag="wtile", name="wm")
                nc.sync.dma_start(wm[:], w_d.ap()[m * P:(m + 1) * P, :])
            pss = [psy.tile([P, ch], F32, tag="psy", name=f"psy{_}")
                   for _ in range(nch)]
            for k in range(kt):
                for c in range(nch):
                    nc.tensor.matmul(pss[c][:], wm[:, k * P:(k + 1) * P],
                                     tin[k][:, c * ch:(c + 1) * ch],
                                     start=(k == 0), stop=(k == kt - 1))
            if m == 1 and mid_fn is not None:
                mid_fn()
            ty = out_pool.tile([P, tokpc], BF16, tag=f"{tag}{m}",
                               name=f"{tag}{m}")
            for c in range(nch):
                nc.scalar.activation(ty[:, c * ch:(c + 1) * ch], pss[c][:],
                                     AF.Identity)
                if m == 0:
                    nc.scalar.activation(accs[c][:], pss[c][:], AF.Square)
                    nc.vector.memset(accs[c][0:1, :], 0.0)
                else:
                    sq = sqp.tile([P, ch], F32, tag="sq", name="sq")
                    nc.scalar.activation(sq[:], pss[c][:], AF.Square)
                    nc.vector.tensor_tensor(accs[c][:], accs[c][:], sq[:],
                                            ALU.add)
            if m == 0:
                nc.vector.memset(ty[0:1, :], 0.0)
            tout.append(ty)

        def finish():
            ps_norm = [psn.tile([1, ch], F32, tag="psn", name=f"psn{_}")
                       for _ in range(nch)]
            for c in range(nch):
                nc.tensor.matmul(ps_norm[c][:], ones_f[:], accs[c][:],
                                 start=True, stop=True)
            return ps_norm
        return tout, finish

    def boundary(fin, first):
        """Inter-layer clamp folded into cs2 (squared cumulative scale).
        f = min(1, 10/sqrt(cs2_prev*ssq_raw)); cs2_new = cs2_prev * f^2.
        rsqrt via exp(-0.5*ln(.)) - single act table, no DVE reciprocal."""
        ps_norm = fin()
        cs2_new = scalL.tile([1, tokpc], F32, tag="cs2", name="cs2")
        for c in range(nch):
            sl = slice(c * ch, (c + 1) * ch)
            if first:
                src = ps_norm[c][:]
            else:
                m2 = stile_s()
                nc.vector.tensor_tensor(m2[:], ps_norm[c][:],
                                        S["cs2"][:, sl], ALU.mult)
                src = m2[:]
            L = stile_s()
            nc.scalar.activation(L[:], src, AF.Ln, bias=c_eps2[0:1, :])
            r10 = stile_s()
            nc.scalar.activation(r10[:], L[:], AF.Exp, scale=-0.5,
                                 bias=c_ln10[0:1, :])
            f = stile_s()
            nc.vector.tensor_scalar_min(f[:], r10[:], 1.0)
            if first:
                nc.vector.tensor_tensor(cs2_new[:, sl], f[:], f[:], ALU.mult)
            else:
                ff = stile_s()
                nc.vector.tensor_tensor(ff[:], f[:], f[:], ALU.mult)
                nc.vector.tensor_tensor(cs2_new[:, sl], S["cs2"][:, sl],
                                        ff[:], ALU.mult)
        S["cs2"] = cs2_new

    # ---------- Layers 1, 2 ----------
    t1p = tc.alloc_tile_pool(name="t1", bufs=1, side="left")
    t1, fin1 = gemm_layer(t0, C["w1"], C["kt1"], C["mt1"], t1p, "t1_",
                          preloaded=w1_pre)
    t0p.release()

    t2p = tc.alloc_tile_pool(name="t2", bufs=1, side="right")
    t2, fin2 = gemm_layer(t1, C["w2"], C["kt2"], C["mt2"], t2p, "t2_",
                          mid_fn=lambda: boundary(fin1, first=True))
    t1p.release()

    # ---------- Layer 3 + expmap0/projx, chunk-split so chunk 0's tail
    # overlaps chunk 1's matmuls ----------
    kt, mt = C["kt3"], C["mt3"]
    y3p = tc.alloc_tile_pool(name="y3", bufs=1, side="left")
    y3 = [y3p.tile([P, tokpc], F32, tag=f"y3_{m}", name=f"y3_{m}")
          for m in range(mt)]
    deferred_tail = None
    for c in range(nch):
        sl = slice(c * ch, (c + 1) * ch)
        acc3 = accp.tile([P, ch], F32, tag="acc", name=f"acc3_{c}")
        for m in range(mt):
            wm = wp.tile([P, kt * P], BF16, tag="wtile", name="wm")
            nc.sync.dma_start(wm[:], C["w3"].ap()[m * P:(m + 1) * P, :])
            ps = psy.tile([P, ch], F32, tag="psy", name="psy3")
            for k in range(kt):
                nc.tensor.matmul(ps[:], wm[:, k * P:(k + 1) * P],
                                 t2[k][:, sl],
                                 start=(k == 0), stop=(k == kt - 1))
            if m == 1:
                if c == 0:
                    boundary(fin2, first=False)
                if deferred_tail is not None:
                    deferred_tail()
                    deferred_tail = None
            nc.scalar.activation(y3[m][:, sl], ps[:], AF.Identity)
            if m == 0:
                nc.scalar.activation(acc3[:], ps[:], AF.Square)
                nc.vector.memset(acc3[0:1, :], 0.0)
            else:
                sq = sqp.tile([P, ch], F32, tag="sq", name="sq")
                nc.scalar.activation(sq[:], ps[:], AF.Square)
                nc.vector.tensor_tensor(acc3[:], acc3[:], sq[:], ALU.add)

        def chunk_tail(sl=sl, acc3=acc3, c=c):
            # true norm n = sqrt(cs2*ssq_raw); nc = min(n, 10)
            # out[0] = cosh(nc) = 0.5 e^nc + 0.5 e^-nc
            # out[1:] = y3_raw * sinh(nc) / sqrt(ssq_raw)
            ps_norm = psn.tile([1, ch], F32, tag="psn", name=f"psn3_{c}")
            nc.tensor.matmul(ps_norm[:], ones_f[:], acc3[:],
                             start=True, stop=True)
            # raw-rsqrt branch (independent of the cs2 branch)
            Lr = stile_s()
            nc.scalar.activation(Lr[:], ps_norm[:], AF.Ln, bias=c_eps2[0:1, :])
            rr = stile_s()
            nc.scalar.activation(rr[:], Lr[:], AF.Exp, scale=-0.5)
            # true-norm branch
            m2 = stile_s()
            nc.vector.tensor_tensor(m2[:], ps_norm[:], S["cs2"][:, sl],
                                    ALU.mult)
            Lm = stile_s()
            nc.scalar.activation(Lm[:], m2[:], AF.Ln, bias=c_eps2[0:1, :])
            n_ = stile_s()
            nc.scalar.activation(n_[:], Lm[:], AF.Exp, scale=0.5)
            ncl = stile_s()
            nc.vector.tensor_scalar_min(ncl[:], n_[:], MAX_TAN_NORM)
            e_ = stile_s()
            nc.scalar.activation(e_[:], ncl[:], AF.Exp, scale=1.0,
                                 bias=c_lnhalf[0:1, :])
            en = stile_s()
            nc.scalar.activation(en[:], ncl[:], AF.Exp, scale=-1.0,
                                 bias=c_lnhalf[0:1, :])
            cosh_c = stile_s()
            nc.vector.tensor_tensor(cosh_c[:], e_[:], en[:], ALU.add)
            sh = stile_s()
            nc.vector.tensor_tensor(sh[:], e_[:], en[:], ALU.subtract)
            s3 = stile_s()
            nc.vector.tensor_tensor(s3[:], sh[:], rr[:], ALU.mult)
            s3b = bcast.tile([P, ch], F32, tag="s3b", name="s3b")
            nc.gpsimd.partition_broadcast(s3b[:], s3[:])
            for m in range(mt):
                ot = outp.tile([P, ch], F32, tag="ot", name="ot")
                nc.vector.tensor_tensor(ot[:], y3[m][:, sl], s3b[:],
                                        ALU.mult)
                if m == 0:
                    nc.vector.tensor_copy(ot[0:1, :], cosh_c[:])
                nc.sync.dma_start(C["out"].ap()[m * P:(m + 1) * P, sl],
                                  ot[:])

        deferred_tail = chunk_tail
    deferred_tail()
    t2p.release()
    y3p.release()

    for p in (outp, psn, psy, wp, accp, sqp, bcast, scalS, scalL, const):
        p.release()


# =====================================================================
# General (nonzero-bias) fallback program - barrier between layers
# =====================================================================

def build_nc_general(tokpc=TOKPC, din=D_IN, dhid=D_HID, dout=D_OUT, ch=512):
    assert tokpc % ch == 0
    nch = tokpc // ch
    kt1, mt1 = din // P, dhid // P
    kt2, mt2 = dhid // P, dhid // P
    kt3, mt3 = dhid // P, dout // P

    nc = bacc.Bacc("TRN2", target_bir_lowering=False, debug=False,
                   num_devices=N_CORES)

    xt_d = nc.dram_tensor("xt", [din, tokpc], BF16, kind="ExternalInput")
    x0_d = nc.dram_tensor("x0", [1, tokpc], F32, kind="ExternalInput")
    w1_d = nc.dram_tensor("w1", [mt1 * P, din], BF16, kind="ExternalInput")
    w2_d = nc.dram_tensor("w2", [mt2 * P, dhid], BF16, kind="ExternalInput")
    w3_d = nc.dram_tensor("w3", [mt3 * P, dhid], BF16, kind="ExternalInput")
    b1_d = nc.dram_tensor("b1", [P, mt1], F32, kind="ExternalInput")
    b2_d = nc.dram_tensor("b2", [P, mt2], F32, kind="ExternalInput")
    b3_d = nc.dram_tensor("b3", [P, mt3], F32, kind="ExternalInput")
    out_d = nc.dram_tensor("out", [dout, tokpc], F32, kind="ExternalOutput")

    with tile.TileContext(nc) as tc:
        _build_general_program(tc, nc, dict(
            tokpc=tokpc, din=din, dhid=dhid, dout=dout, ch=ch, nch=nch,
            kt1=kt1, mt1=mt1, kt2=kt2, mt2=mt2, kt3=kt3, mt3=mt3,
            xt=xt_d, x0=x0_d, w1=w1_d, w2=w2_d, w3=w3_d,
            b1=b1_d, b2=b2_d, b3=b3_d, out=out_d,
        ))
    nc.compile()
    return nc


def _build_general_program(tc, nc, C):
    tokpc, ch, nch = C["tokpc"], C["ch"], C["nch"]

    const = tc.alloc_tile_pool(name="const", bufs=1)
    scalL = tc.alloc_tile_pool(name="scalL", bufs=5)
    scalS = tc.alloc_tile_pool(name="scalS", bufs=6)
    bcast = tc.alloc_tile_pool(name="bcast", bufs=2)
    sqp = tc.alloc_tile_pool(name="sq", bufs=2)
    accp = tc.alloc_tile_pool(name="acc", bufs=4)
    wp = tc.alloc_tile_pool(name="wt", bufs=3)
    psy = tc.alloc_tile_pool(name="psy", bufs=4, space="PSUM")
    psn = tc.alloc_tile_pool(name="psn", bufs=4, space="PSUM")
    outp = tc.alloc_tile_pool(name="outp", bufs=4)

    bias1 = const.tile([P, C["mt1"]], F32, tag="bias1")
    nc.sync.dma_start(bias1[:], C["b1"].ap())
    bias2 = const.tile([P, C["mt2"]], F32, tag="bias2")
    nc.sync.dma_start(bias2[:], C["b2"].ap())
    bias3 = const.tile([P, C["mt3"]], F32, tag="bias3")
    nc.sync.dma_start(bias3[:], C["b3"].ap())
    ones_f = const.tile([P, 1], F32, tag="ones_f", name="ones_f")
    nc.vector.memset(ones_f[:], 1.0)

    def stile_l():
        return scalL.tile([1, tokpc], F32, tag="sl", name="sl")

    def stile_s():
        return scalS.tile([1, ch], F32, tag="ss", name="ss")

    def norm_accum_tiles():
        return [psn.tile([1, ch], F32, tag="psn", name=f"psn{_}")
                for _ in range(nch)]

    def bcast_full(s_full):
        sb = bcast.tile([P, tokpc], F32, tag="sb", name="sb")
        nc.gpsimd.partition_broadcast(sb[:], s_full[:])
        return sb

    def gemm_layer(tin, w_d, bias_t, kt, mt, out_pool, out_dtype, tag):
        accs = [accp.tile([P, ch], F32, tag="acc", name=f"acc{_}")
                for _ in range(nch)]
        tout = []
        for m in range(mt):
            wm = wp.tile([P, kt * P], BF16, tag="wtile", name="wm")
            nc.sync.dma_start(wm[:], w_d.ap()[m * P:(m + 1) * P, :])
            pss = [psy.tile([P, ch], F32, tag="psy", name=f"psy{_}")
                   for _ in range(nch)]
            for k in range(kt):
                for c in range(nch):
                    nc.tensor.matmul(pss[c][:], wm[:, k * P:(k + 1) * P],
                                     tin[k][:, c * ch:(c + 1) * ch],
                                     start=(k == 0), stop=(k == kt - 1))
            ty = out_pool.tile([P, tokpc], out_dtype, tag=f"{tag}{m}",
                               name=f"{tag}{m}")
            for c in range(nch):
                nc.scalar.activation(ty[:, c * ch:(c + 1) * ch], pss[c][:],
                                     AF.Identity, bias=bias_t[:, m:m + 1],
                                     scale=1.0)
                if m == 0:
                    nc.scalar.activation(accs[c][:], pss[c][:], AF.Square,
                                         bias=bias_t[:, m:m + 1], scale=1.0)
                    nc.vector.memset(accs[c][0:1, :], 0.0)
                else:
                    sq = sqp.tile([P, ch], F32, tag="sq", name="sq")
                    nc.scalar.activation(sq[:], pss[c][:], AF.Square,
                                         bias=bias_t[:, m:m + 1], scale=1.0)
                    nc.vector.tensor_tensor(accs[c][:], accs[c][:], sq[:],
                                            ALU.add)
            if m == 0:
                nc.vector.memset(ty[0:1, :], 0.0)
            tout.append(ty)

        def finish():
            ps_norm = norm_accum_tiles()
            for c in range(nch):
                nc.tensor.matmul(ps_norm[c][:], ones_f[:], accs[c][:],
                                 start=True, stop=True)
            return ps_norm
        return tout, finish

    def clamp_scale(ps_norm):
        """s = min(max(sqrt(ssq),eps),10)/max(sqrt(ssq),eps) via ln/exp."""
        s = stile_l()
        for c in range(nch):
            L = stile_s()
            nc.scalar.activation(L[:], ps_norm[c][:], AF.Ln, bias=EPS2)
            r10 = stile_s()
            nc.scalar.activation(r10[:], L[:], AF.Exp, scale=-0.5, bias=LN10)
            f = stile_s()
            nc.vector.tensor_scalar_min(f[:], r10[:], 1.0)
            nc.vector.tensor_copy(s[:, c * ch:(c + 1) * ch], f[:])
        return s

    def apply_scale(tiles, sb):
        for t in tiles:
            nc.vector.tensor_tensor(t[:], t[:], sb[:], ALU.mult)

    def body():
        # Phase 0: load bf16 xs (= raw t0), input norm, s0
        t0p = tc.alloc_tile_pool(name="t0", bufs=1, side="right")
        t0 = []
        for k in range(C["kt1"]):
            t = t0p.tile([P, tokpc], BF16, tag=f"t0_{k}", name=f"t0_{k}")
            nc.sync.dma_start(t[:], C["xt"].ap()[k * P:(k + 1) * P, :])
            t0.append(t)
        x0t = stile_l()
        nc.sync.dma_start(x0t[:], C["x0"].ap())

        acc0 = [accp.tile([P, ch], F32, tag="acc", name=f"acc0_{_}")
                for _ in range(nch)]
        for k in range(C["kt1"]):
            for c in range(nch):
                if k == 0:
                    nc.scalar.activation(acc0[c][:],
                                         t0[k][:, c * ch:(c + 1) * ch],
                                         AF.Square)
                else:
                    sq = sqp.tile([P, ch], F32, tag="sq", name="sq")
                    nc.scalar.activation(sq[:], t0[k][:, c * ch:(c + 1) * ch],
                                         AF.Square)
                    nc.vector.tensor_tensor(acc0[c][:], acc0[c][:], sq[:],
                                            ALU.add)

        # norm-MM for the input + s0 = arccosh(x0) / ||xs|| chain
        # (x0 input here is pre-clipped arccosh distance d, computed on host)
        ps_n0 = norm_accum_tiles()
        for c in range(nch):
            nc.tensor.matmul(ps_n0[c][:], ones_f[:], acc0[c][:],
                             start=True, stop=True)
        s0 = stile_l()
        for c in range(nch):
            L = stile_s()
            nc.scalar.activation(L[:], ps_n0[c][:], AF.Ln, bias=EPS2)
            r = stile_s()
            nc.scalar.activation(r[:], L[:], AF.Exp, scale=-0.5)
            nc.vector.tensor_tensor(s0[:, c * ch:(c + 1) * ch],
                                    x0t[:, c * ch:(c + 1) * ch], r[:],
                                    ALU.mult)

        s0b = bcast_full(s0)
        for k in range(C["kt1"]):
            nc.vector.tensor_tensor(t0[k][:], t0[k][:], s0b[:], ALU.mult)

        # Layers 1, 2 with barrier scale application
        t1p = tc.alloc_tile_pool(name="t1", bufs=1, side="left")
        t1, fin1 = gemm_layer(t0, C["w1"], bias1, C["kt1"], C["mt1"],
                              t1p, BF16, "t1_")
        apply_scale(t1, bcast_full(clamp_scale(fin1())))
        t0p.release()

        t2p = tc.alloc_tile_pool(name="t2", bufs=1, side="right")
        t2, fin2 = gemm_layer(t1, C["w2"], bias2, C["kt2"], C["mt2"],
                              t2p, BF16, "t2_")
        apply_scale(t2, bcast_full(clamp_scale(fin2())))
        t1p.release()

        # Layer 3 + expmap0/projx
        kt, mt = C["kt3"], C["mt3"]
        y3p = tc.alloc_tile_pool(name="y3", bufs=1, side="left")
        y3 = [y3p.tile([P, tokpc], F32, tag=f"y3_{m}", name=f"y3_{m}")
              for m in range(mt)]
        deferred_tail = None
        for c in range(nch):
            sl = slice(c * ch, (c + 1) * ch)
            acc3 = accp.tile([P, ch], F32, tag="acc", name=f"acc3_{c}")
            for m in range(mt):
                wm = wp.tile([P, kt * P], BF16, tag="wtile", name="wm")
                nc.sync.dma_start(wm[:], C["w3"].ap()[m * P:(m + 1) * P, :])
                ps = psy.tile([P, ch], F32, tag="psy", name="psy3")
                for k in range(kt):
                    nc.tensor.matmul(ps[:], wm[:, k * P:(k + 1) * P],
                                     t2[k][:, sl],
                                     start=(k == 0), stop=(k == kt - 1))
                if m == 1 and deferred_tail is not None:
                    deferred_tail()
                    deferred_tail = None
                nc.scalar.activation(y3[m][:, sl], ps[:], AF.Identity,
                                     bias=bias3[:, m:m + 1], scale=1.0)
                if m == 0:
                    nc.scalar.activation(acc3[:], ps[:], AF.Square,
                                         bias=bias3[:, m:m + 1], scale=1.0)
                    nc.vector.memset(acc3[0:1, :], 0.0)
                else:
                    sq = sqp.tile([P, ch], F32, tag="sq", name="sq")
                    nc.scalar.activation(sq[:], ps[:], AF.Square,
                                         bias=bias3[:, m:m + 1], scale=1.0)
                    nc.vector.tensor_tensor(acc3[:], acc3[:], sq[:], ALU.add)

            def chunk_tail(sl=sl, acc3=acc3, c=c):
                ps_norm = psn.tile([1, ch], F32, tag="psn", name=f"psn3_{c}")
                nc.tensor.matmul(ps_norm[:], ones_f[:], acc3[:],
                                 start=True, stop=True)
                Lr = stile_s()
                nc.scalar.activation(Lr[:], ps_norm[:], AF.Ln, bias=EPS2)
                rr = stile_s()
                nc.scalar.activation(rr[:], Lr[:], AF.Exp, scale=-0.5)
                n_ = stile_s()
                nc.scalar.activation(n_[:], Lr[:], AF.Exp, scale=0.5)
                ncl = stile_s()
                nc.vector.tensor_scalar_min(ncl[:], n_[:], MAX_TAN_NORM)
                e_ = stile_s()
                nc.scalar.activation(e_[:], ncl[:], AF.Exp, scale=1.0,
                                     bias=LNHALF)
                en = stile_s()
                nc.scalar.activation(en[:], ncl[:], AF.Exp, scale=-1.0,
                                     bias=LNHALF)
                cosh_c = stile_s()
                nc.vector.tensor_tensor(cosh_c[:], e_[:], en[:], ALU.add)
                sh = stile_s()
                nc.vector.tensor_tensor(sh[:], e_[:], en[:], ALU.subtract)
                s3 = stile_s()
                nc.vector.tensor_tensor(s3[:], sh[:], rr[:], ALU.mult)
                s3b = bcast.tile([P, ch], F32, tag="s3b", name="s3b")
                nc.gpsimd.partition_broadcast(s3b[:], s3[:])
                for m in range(mt):
                    ot = outp.tile([P, ch], F32, tag="ot", name="ot")
                    nc.vector.tensor_tensor(ot[:], y3[m][:, sl], s3b[:],
                                            ALU.mult)
                    if m == 0:
                        nc.vector.tensor_copy(ot[0:1, :], cosh_c[:])
                    nc.sync.dma_start(C["out"].ap()[m * P:(m + 1) * P, sl],
                                      ot[:])

            deferred_tail = chunk_tail
        deferred_tail()
        t2p.release()
        y3p.release()

    body()
    for p in (outp, psn, psy, wp, accp, sqp, bcast, scalS, scalL, const):
        p.release()


# =====================================================================
# host-side prep + entry point
# =====================================================================

def _block_weight(w):
    """W [dout, din] f32 -> [mt*128, din] bf16 with row m*128+p holding, for
    each k-tile, lhsT tile (k,m) row p: out[m*128+p, k*128+j] = W.T[k*128+p,
    m*128+j].  One fully-contiguous [128, kt*128] DMA per m-tile."""
    dout, din = w.shape
    mt, kt = dout // P, din // P
    w = np.asarray(w, dtype=np.float32)
    blocked = (w.reshape(mt, P, kt, P)       # [m, j, k, p]
                .transpose(0, 3, 2, 1)       # [m, p, k, j]
                .reshape(mt * P, din))
    return np.ascontiguousarray(blocked.astype(ml_dtypes.bfloat16))


def _prep_bias(b, mt):
    """b [d] -> [128, mt] f32 with out[p, m] = b[m*128+p]."""
    return np.ascontiguousarray(
        np.asarray(b, dtype=np.float32).reshape(mt, P).T)


@functools.lru_cache(maxsize=2)
def _get_nc(fold=True):
    return build_nc_fold() if fold else build_nc_general()


def prep_in_maps_fold(x_hyp, W1, W2, W3):
    """logmap0 on the host: t0 = [0, d*xs/||xs||] feature-major bf16."""
    w1b = _block_weight(W1)
    w2b = _block_weight(W2)
    w3b = _block_weight(W3)
    x = np.asarray(x_hyp, dtype=np.float32)
    d_all = np.arccosh(np.maximum(x[:, 0], 1.0 + EPS))          # [TOK]
    xs_all = x[:, 1:]                                           # [TOK, 1023]
    ns_all = np.maximum(np.linalg.norm(xs_all, axis=1), EPS)
    s0_all = (d_all / ns_all).astype(np.float32)                # [TOK]
    in_maps = []
    for c in range(N_CORES):
        rows = slice(c * TOKPC, (c + 1) * TOKPC)
        t0 = np.zeros((D_IN, TOKPC), dtype=np.float32)
        t0[1:, :] = (xs_all[rows] * s0_all[rows, None]).T
        xt = t0.astype(ml_dtypes.bfloat16)
        in_maps.append(dict(xt=np.ascontiguousarray(xt),
                            w1=w1b, w2=w2b, w3=w3b))
    return in_maps


def prep_in_maps_general(x_hyp, W1, b1, W2, b2, W3, b3):
    w1b = _block_weight(W1)
    w2b = _block_weight(W2)
    w3b = _block_weight(W3)
    b1c = _prep_bias(b1, D_HID // P)
    b2c = _prep_bias(b2, D_HID // P)
    b3c = _prep_bias(b3, D_OUT // P)
    x = np.asarray(x_hyp, dtype=np.float32)
    in_maps = []
    for c in range(N_CORES):
        shard = x[c * TOKPC:(c + 1) * TOKPC, :]  # [tokpc, din]
        xt = shard.T.astype(ml_dtypes.bfloat16)
        xt[0, :] = 0  # zero time component (norm + GEMM both want it out)
        # x0 slot carries d = arccosh(clip(x0)) precomputed on host
        d = np.arccosh(np.maximum(shard[:, 0:1], 1.0 + EPS)).T
        in_maps.append(dict(xt=np.ascontiguousarray(xt),
                            x0=np.ascontiguousarray(d.astype(np.float32)),
                            w1=w1b, w2=w2b, w3=w3b,
                            b1=b1c, b2=b2c, b3=b3c))
    return in_maps


LAST_RESULTS = None


def kernel(x_hyp, W1, b1, W2, b2, W3, b3):
    global LAST_RESULTS
    fold = not (np.any(b1) or np.any(b2) or np.any(b3))
    nc = _get_nc(fold)
    if fold:
        in_maps = prep_in_maps_fold(x_hyp, W1, W2, W3)
    else:
        in_maps = prep_in_maps_general(x_hyp, W1, b1, W2, b2, W3, b3)
    res = bass_utils.run_bass_kernel_spmd(nc, in_maps,
                                          core_ids=list(range(N_CORES)))
    LAST_RESULTS = res
    parts = [np.asarray(res.results[c]["out"]).T for c in range(N_CORES)]
    return np.ascontiguousarray(np.concatenate(parts, axis=0),
                                dtype=np.float32)
